# revision 1
# baseline (speedup 1.0000x reference)
"""2D Daubechies-2 DWT (single level) on Trainium2, 8-core data parallel.

Input  x: [16, 1024, 1024, 1] f32  ->  Output: [16, 512, 512, 4] f32
Channels: [LL, LH, HL, HH] = [(wL,hL), (wL,hH), (wH,hL), (wH,hH)].

Per core: 2 images, ~58 device instructions.

Key layout trick: output row index i = 4*p + j (p = partition, j in [0,4)),
so the column pass's source rows h = 2i + delta = 8p + (2j + delta) live inside
partition p's own 8-row band for delta >= 0 — the whole image loads with ONE
contiguous DMA ([p] <- rows 8p..8p+7), taps are free-dim offsets, and only the
j=0 taps of delta<0 need a small one-row-per-partition gather (plus the
symmetric-mirror rows for i=0). Both wavelet passes are Vector-engine FMA
chains (tensor_scalar / scalar_tensor_tensor) that ping-pong between scratch
tiles (never aliasing in1 with out — aliased accumulation is pathologically
slow here). The row pass reads stride-2 free-dim taps from a pitch-1026
mirror-prefixed intermediate and writes the output with channels interleaved;
the store is one fully contiguous DMA per image.
"""
import math

import numpy as np

import concourse.bass as bass
import concourse.tile as tile
from concourse import bacc, mybir
from concourse.bass_utils import run_bass_kernel_spmd

N_CORES = 8
IMGS = 2
IMG_ELEMS = 1024 * 1024
OUT_ELEMS = 512 * 512 * 4
F32 = mybir.dt.float32
AO = mybir.AluOpType
PITCH = 1026  # 2 mirror-prefix cols + 1024 data cols

_S3 = math.sqrt(3.0)
_DEN = 4.0 * math.sqrt(2.0)
H4 = [np.float32((1 + _S3) / _DEN), np.float32((3 + _S3) / _DEN),
      np.float32((3 - _S3) / _DEN), np.float32((1 - _S3) / _DEN)]
G4 = [H4[3], np.float32(-H4[2]), H4[1], np.float32(-H4[0])]
FILT = {"L": H4, "H": G4}


def _ap(handle, offset, dims):
    return bass.AP(handle, offset, [list(d) for d in dims])


def _tap(t, off, dims, pcnt=128, poff=0):
    f = t[:]
    pitch = f.ap[0][0]
    return bass.AP(f.tensor, f.offset + poff * pitch + off,
                   [[pitch, pcnt]] + [list(d) for d in dims])


def _build(reps=1):
    nc = bacc.Bacc("TRN2", target_bir_lowering=False, debug=False, num_devices=1)
    xh = nc.dram_tensor("x", [IMGS * IMG_ELEMS], F32, kind="ExternalInput")
    yh = nc.dram_tensor("y", [IMGS * OUT_ELEMS], F32, kind="ExternalOutput")

    with tile.TileContext(nc) as tc:
        with (
            tc.tile_pool(name="t32", bufs=2) as p32,
            tc.tile_pool(name="t16", bufs=4) as p16,
            tc.tile_pool(name="lh", bufs=1) as plh,
        ):
            for _rep in range(reps):
                # LH: [p, (f 2)(img 2)(j 4)(PITCH)]
                LH = plh.tile([128, 2 * 2 * 4 * PITCH], F32, tag="lh")

                # one merged gather for the j=0 taps of both delta<0 and both
                # images: rows 8p-2,8p-1 are one contiguous 8KB run per p.
                # XS layout: [p, (img 2)(slot 2: d=-2,-1)(1024)]
                XS = p16.tile([128, 4096], F32, tag="t16")
                nc.gpsimd.dma_start(
                    _tap(XS, 0, [[2048, 2], [1, 2048]], pcnt=127, poff=1),
                    _ap(xh, 6 * 1024, [[8192, 127], [1048576, 2], [1, 2048]]))
                # partition 0 mirrors: slot d=-2 <- row 1, slot d=-1 <- row 0
                nc.gpsimd.dma_start(
                    _tap(XS, 0, [[2048, 2], [1, 1024]], pcnt=1),
                    _ap(xh, 1024, [[1, 1], [1048576, 2], [1, 1024]]))
                nc.gpsimd.dma_start(
                    _tap(XS, 1024, [[2048, 2], [1, 1024]], pcnt=1),
                    _ap(xh, 0, [[1, 1], [1048576, 2], [1, 1024]]))

                X8s = []
                for img in range(IMGS):
                    x8 = p32.tile([128, 8192], F32, tag="t32")
                    dma = nc.sync.dma_start if img == 0 else nc.gpsimd.dma_start
                    dma(x8[:], _ap(xh, img * IMG_ELEMS, [[8192, 128], [1, 8192]]))
                    X8s.append(x8)
                for img in range(IMGS):
                    X8 = X8s[img]
                    xs0 = {-2: _tap(XS, img * 2048, [[1, 1024]]),
                           -1: _tap(XS, img * 2048 + 1024, [[1, 1024]])}

                    # column-pass FMA chains, per filter (acc FD = (j 4)(1024))
                    for fi, f in enumerate(("L", "H")):
                        c0, c1, c2, c3 = (float(FILT[f][k]) for k in range(4))
                        a1 = p16.tile([128, 4096], F32, tag="t16")
                        # k=0 (delta=-2): j=0 from xs0[-2]; j=1..3 bands 0,2,4
                        nc.vector.tensor_scalar_mul(
                            _tap(a1, 0, [[1, 1024]]), xs0[-2], c0)
                        nc.vector.tensor_scalar_mul(
                            _tap(a1, 1024, [[1024, 3], [1, 1024]]),
                            _tap(X8, 0, [[2048, 3], [1, 1024]]), c0)
                        # k=1 (delta=-1): j=0 from xs0[-1]; j=1..3 bands 1,3,5
                        a2 = p16.tile([128, 4096], F32, tag="t16")
                        nc.vector.scalar_tensor_tensor(
                            out=_tap(a2, 0, [[1, 1024]]), in0=xs0[-1], scalar=c1,
                            in1=_tap(a1, 0, [[1, 1024]]), op0=AO.mult, op1=AO.add)
                        nc.vector.scalar_tensor_tensor(
                            out=_tap(a2, 1024, [[1024, 3], [1, 1024]]),
                            in0=_tap(X8, 1024, [[2048, 3], [1, 1024]]), scalar=c1,
                            in1=_tap(a1, 1024, [[1024, 3], [1, 1024]]),
                            op0=AO.mult, op1=AO.add)
                        # k=2 (delta=0): bands 0,2,4,6 — all j
                        a3 = p16.tile([128, 4096], F32, tag="t16")
                        nc.vector.scalar_tensor_tensor(
                            out=_tap(a3, 0, [[1024, 4], [1, 1024]]),
                            in0=_tap(X8, 0, [[2048, 4], [1, 1024]]), scalar=c2,
                            in1=_tap(a2, 0, [[1024, 4], [1, 1024]]),
                            op0=AO.mult, op1=AO.add)
                        # k=3 (delta=1): bands 1,3,5,7 -> LH slice (data at +2)
                        nc.vector.scalar_tensor_tensor(
                            out=_tap(LH, fi * 8 * PITCH + img * 4 * PITCH + 2,
                                     [[PITCH, 4], [1, 1024]]),
                            in0=_tap(X8, 1024, [[2048, 4], [1, 1024]]), scalar=c3,
                            in1=_tap(a3, 0, [[1024, 4], [1, 1024]]),
                            op0=AO.mult, op1=AO.add)

                # row-pass mirror prefix: col0 <- w=1 (data col 3), col1 <- w=0
                nc.vector.tensor_copy(_tap(LH, 0, [[PITCH, 16], [1, 1]]),
                                      _tap(LH, 3, [[PITCH, 16], [1, 1]]))
                nc.vector.tensor_copy(_tap(LH, 1, [[PITCH, 16], [1, 1]]),
                                      _tap(LH, 2, [[PITCH, 16], [1, 1]]))

                # row pass: per (f_in, f_out) FMA chain over stride-2 taps
                Oh = []
                for _img in range(IMGS):
                    oimg = p32.tile([128, 8192], F32, tag="t32")
                    Oh.append(oimg)
                for fi in range(2):
                    for fo, f_out in enumerate(("L", "H")):
                        ch = 2 * fo + fi
                        cs = [float(FILT[f_out][k]) for k in range(4)]
                        acc = None
                        for k in range(3):
                            src = _tap(LH, fi * 8 * PITCH + k,
                                       [[4 * PITCH, 2], [PITCH, 4], [2, 512]])
                            if k == 0:
                                acc = p16.tile([128, 4096], F32, tag="t16")
                                nc.vector.tensor_scalar_mul(
                                    _tap(acc, 0, [[2048, 2], [512, 4], [1, 512]]),
                                    src, cs[0])
                            else:
                                nxt = p16.tile([128, 4096], F32, tag="t16")
                                nc.vector.scalar_tensor_tensor(
                                    out=_tap(nxt, 0, [[2048, 2], [512, 4], [1, 512]]),
                                    in0=src, scalar=cs[k],
                                    in1=_tap(acc, 0, [[2048, 2], [512, 4], [1, 512]]),
                                    op0=AO.mult, op1=AO.add)
                                acc = nxt
                        for img in range(IMGS):
                            nc.vector.scalar_tensor_tensor(
                                out=_tap(Oh[img], ch, [[2048, 4], [4, 512]]),
                                in0=_tap(LH, fi * 8 * PITCH + img * 4 * PITCH + 3,
                                         [[PITCH, 4], [2, 512]]),
                                scalar=cs[3],
                                in1=_tap(acc, img * 2048, [[512, 4], [1, 512]]),
                                op0=AO.mult, op1=AO.add)

                # store: i = 4p + j  ->  fully contiguous per image
                for img in range(IMGS):
                    dma = nc.sync.dma_start if img == 0 else nc.gpsimd.dma_start
                    dma(_ap(yh, img * OUT_ELEMS, [[8192, 128], [1, 8192]]),
                        Oh[img][:])
    nc.compile()
    return nc


_NC_CACHE = {}


def _get_nc(reps=1):
    if reps not in _NC_CACHE:
        _NC_CACHE[reps] = _build(reps)
    return _NC_CACHE[reps]


def kernel(**inputs):
    x = np.asarray(inputs["x"], dtype=np.float32)
    assert x.shape == (16, 1024, 1024, 1), x.shape
    nc = _get_nc(1)
    xs = np.ascontiguousarray(x.reshape(N_CORES, IMGS * IMG_ELEMS))
    in_maps = [{"x": xs[i]} for i in range(N_CORES)]
    res = run_bass_kernel_spmd(nc, in_maps, core_ids=list(range(N_CORES)))
    out = np.stack([res.results[i]["y"].reshape(IMGS, 512, 512, 4)
                    for i in range(N_CORES)])
    return out.reshape(16, 512, 512, 4)


def run_reps(reps, n_calls=3):
    import time
    nc = _get_nc(reps)
    rng = np.random.default_rng(0)
    xs = rng.standard_normal((N_CORES, IMGS * IMG_ELEMS), dtype=np.float32)
    in_maps = [{"x": xs[i]} for i in range(N_CORES)]
    best = float("inf")
    for _ in range(n_calls):
        t0 = time.time()
        run_bass_kernel_spmd(nc, in_maps, core_ids=list(range(N_CORES)))
        best = min(best, time.time() - t0)
    return best



# revision 7
# speedup vs baseline: 37.5844x; 37.5844x over previous
"""2D Daubechies-2 DWT on Trainium2 — all-TensorE design, bf16, 8-core DP.

Input  x: [16, 1024, 1024, 1] f32  ->  Output: [16, 512, 512, 4] f32
Per core: 2 images. Host casts input to bf16 and builds tiny banded filter
matrices; device does both wavelet passes as PE matmuls:

  pass1 (column DWT, contract over h):  MT[w, i] = X_chunk.T @ W_t
    - X chunks [128 h x 128 w] are the STATIONARY operand so the output
      comes out w-major (transposed), which is exactly what pass 2 needs.
    - W is one banded [128 x 130] moving matrix reused for every h-tile
      (cols = 65-wide output window per filter); the t=0 variant bakes the
      symmetric top-mirror into its coefficients. Window overlap columns
      are handled with 1-col accumulate matmuls (start=False).
  drain: PSUM f32 -> SBUF bf16 copies, split ScalarE/VectorE.
  pass2 (row DWT, contract over w): YT[w', i] = R_pat.T @ MT_tile with
    banded stationary R patterns (A0/A/B/C per filter, mirrors baked).
  Output stored planar bf16 [img][c][w'][h']; host transposes to NHWC f32.
"""
import math

import numpy as np
import ml_dtypes

import concourse.bass as bass
import concourse.tile as tile
from concourse import bacc, mybir
from concourse.bass_utils import run_bass_kernel_spmd

N_CORES = 8
IMGS = 2
IMG_ELEMS = 1024 * 1024
OUT_ELEMS = 4 * 512 * 512
F32 = mybir.dt.float32
BF16 = mybir.dt.bfloat16
NPBF16 = ml_dtypes.bfloat16

_S3 = math.sqrt(3.0)
_DEN = 4.0 * math.sqrt(2.0)
H4 = np.array([(1 + _S3) / _DEN, (3 + _S3) / _DEN,
               (3 - _S3) / _DEN, (1 - _S3) / _DEN], dtype=np.float64)
G4 = np.array([H4[3], -H4[2], H4[1], -H4[0]], dtype=np.float64)

P_ORDER = {"A0": 0, "A": 1, "B": 2, "C": 3}


def _make_wmat():
    """[128, 260] f32: cols 0-129 = W0 (t=0, mirror baked), 130-259 = Wn."""
    W = np.zeros((128, 130), dtype=np.float64)
    for c in range(65):
        for k in range(4):
            r = 2 * c + k - 2
            if 0 <= r < 128:
                W[r, c] += H4[k]
                W[r, 65 + c] += G4[k]
    W0 = W.copy()
    W0[1, 0] += H4[0]
    W0[0, 0] += H4[1]
    W0[1, 65] += G4[0]
    W0[0, 65] += G4[1]
    return np.concatenate([W0, W], axis=1).astype(np.float32)


def _make_rmat():
    """[128, 1024] f32: col block (fr*4 + P_ORDER[pat])*128 = pattern."""
    out = np.zeros((128, 1024), dtype=np.float64)
    for fr, coeff in ((0, H4), (1, G4)):
        A = np.zeros((128, 128), dtype=np.float64)
        B = np.zeros((128, 128), dtype=np.float64)
        C = np.zeros((128, 128), dtype=np.float64)
        for c in range(128):
            for k in range(4):
                r = 2 * c + k - 2
                if 0 <= r < 128:
                    A[r, c] += coeff[k]
                rb = 2 * c + k - 130
                if 0 <= rb < 128:
                    B[rb, c] += coeff[k]
                rc = 2 * c + k + 126
                if 0 <= rc < 128:
                    C[rc, c] += coeff[k]
        A0 = A.copy()
        A0[1, 0] += coeff[0]
        A0[0, 0] += coeff[1]
        for name, m in (("A0", A0), ("A", A), ("B", B), ("C", C)):
            out[:, (fr * 4 + P_ORDER[name]) * 128:
                (fr * 4 + P_ORDER[name]) * 128 + 128] = m
    return out.astype(np.float32)


def _ap(handle, offset, dims):
    return bass.AP(handle, offset, [list(d) for d in dims])


def _tap(t, off, dims, pcnt=128, poff=0):
    f = t[:]
    pitch = f.ap[0][0]
    return bass.AP(f.tensor, f.offset + poff * pitch + off,
                   [[pitch, pcnt]] + [list(d) for d in dims])


def _build(reps=1, loop=False, dbg_mt=False):
    nc = bacc.Bacc("TRN2", target_bir_lowering=False, debug=False,
                   num_devices=1)
    xh = nc.dram_tensor("x", [IMGS * IMG_ELEMS], BF16, kind="ExternalInput")
    wh = nc.dram_tensor("wmat", [128 * 260], BF16, kind="ExternalInput")
    rh = nc.dram_tensor("rmat", [128 * 1024], BF16, kind="ExternalInput")
    yh = nc.dram_tensor("y", [IMGS * OUT_ELEMS], BF16, kind="ExternalOutput")

    with tile.TileContext(nc) as tc:
        with (
            tc.tile_pool(name="xs", bufs=2) as px,
            tc.tile_pool(name="mt", bufs=2) as pmt,
            tc.tile_pool(name="yb", bufs=2) as py,
            tc.tile_pool(name="cst", bufs=1) as pc,
            tc.tile_pool(name="pp1", bufs=4, space="PSUM") as pp1,
            tc.tile_pool(name="pp2", bufs=4, space="PSUM") as pp2,
        ):
            def body():
                Wt = pc.tile([128, 260], BF16, tag="wc")
                Rt = pc.tile([128, 1024], BF16, tag="rc")
                nc.sync.dma_start(Wt[:], _ap(wh, 0, [[260, 128], [1, 260]]))
                nc.sync.dma_start(Rt[:], _ap(rh, 0, [[1024, 128], [1, 1024]]))

                for img in range(IMGS):
                    X = px.tile([128, 8192], BF16, tag="xt")
                    nc.sync.dma_start(
                        _tap(X, 0, [[1024, 8], [1, 1024]]),
                        _ap(xh, img * IMG_ELEMS,
                            [[1024, 128], [131072, 8], [1, 1024]]))
                    MT = pmt.tile([128, 8192], BF16, tag="mtt")

                    # ---- pass 1: column DWT -> MT[w, i] ----
                    for c in range(8):
                        psL = pp1.tile([128, 512], F32, tag="p1")
                        psH = pp1.tile([128, 512], F32, tag="p1")
                        for t in range(8):
                            lhsT = _tap(X, t * 1024 + c * 128, [[1, 128]])
                            wofs = 0 if t == 0 else 130
                            n = 64 if t == 7 else 65
                            for f, ps in ((0, psL), (1, psH)):
                                nc.tensor.matmul(
                                    _tap(ps, 64 * t, [[1, n]]),
                                    lhsT,
                                    _tap(Wt, wofs + f * 65, [[1, n]]),
                                    start=(t == 0), stop=(t == 7),
                                    skip_group_check=True)
                        nc.scalar.activation(
                            _tap(MT, c * 1024, [[1, 512]]), psL[:],
                            mybir.ActivationFunctionType.Copy)
                        nc.vector.tensor_copy(
                            _tap(MT, c * 1024 + 512, [[1, 512]]), psH[:])

                    if dbg_mt:
                        nc.gpsimd.dma_start(
                            _ap(yh, img * OUT_ELEMS,
                                [[8192, 128], [1, 8192]]),
                            MT[:])
                        continue

                    # ---- pass 2: row DWT -> YT[w', i] ----
                    Y = py.tile([128, 8192], BF16, tag="yt")
                    for fr in range(2):
                        for m in range(4):
                            b = fr * 4 + m
                            if m == 0:
                                mml = [(0, "A0"), (1, "B")]
                            else:
                                mml = [(2 * m - 1, "C"), (2 * m, "A"),
                                       (2 * m + 1, "B")]
                            ps0 = pp2.tile([128, 512], F32, tag="p2")
                            ps1 = pp2.tile([128, 512], F32, tag="p2")
                            for idx, (t, pat) in enumerate(mml):
                                lhsT = _tap(
                                    Rt, (fr * 4 + P_ORDER[pat]) * 128,
                                    [[1, 128]])
                                st = idx == 0
                                sp = idx == len(mml) - 1
                                for half, ps in ((0, ps0), (1, ps1)):
                                    nc.tensor.matmul(
                                        ps[:], lhsT,
                                        _tap(MT, t * 1024 + half * 512,
                                             [[1, 512]]),
                                        start=st, stop=sp)
                            nc.scalar.activation(
                                _tap(Y, b * 1024, [[1, 512]]), ps0[:],
                                mybir.ActivationFunctionType.Copy)
                            nc.vector.tensor_copy(
                                _tap(Y, b * 1024 + 512, [[1, 512]]), ps1[:])

                        # store planar [c][w'][h'] as soon as this row
                        # filter's 4 blocks are drained (one DMA/channel)
                        for fc in range(2):
                            ch = 2 * fr + fc
                            nc.gpsimd.dma_start(
                                _ap(yh, img * OUT_ELEMS + ch * 262144,
                                    [[512, 128], [65536, 4], [1, 512]]),
                                _tap(Y, fr * 4096 + fc * 512,
                                     [[1024, 4], [1, 512]]))

            if loop and reps > 1:
                with tc.For_i(0, reps, 1):
                    body()
            else:
                for _rep in range(reps):
                    body()
    nc.compile()
    return nc


_NC_CACHE = {}


def _get_nc(reps=1, loop=False):
    key = (reps, loop)
    if key not in _NC_CACHE:
        _NC_CACHE[key] = _build(reps, loop)
    return _NC_CACHE[key]


def _const_maps():
    w = _make_wmat().astype(NPBF16).ravel()
    r = _make_rmat().astype(NPBF16).ravel()
    return w, r


def kernel(**inputs):
    x = np.asarray(inputs["x"], dtype=np.float32)
    assert x.shape == (16, 1024, 1024, 1), x.shape
    nc = _get_nc(1)
    xb = x.reshape(N_CORES, IMGS * IMG_ELEMS).astype(NPBF16)
    w, r = _const_maps()
    in_maps = [{"x": xb[i], "wmat": w, "rmat": r} for i in range(N_CORES)]
    res = run_bass_kernel_spmd(nc, in_maps, core_ids=list(range(N_CORES)))
    # y planar [img][c][w'][h'] bf16 -> [16, h', w', c] f32
    full = np.stack([np.asarray(res.results[i]["y"]).reshape(
        IMGS, 4, 512, 512) for i in range(N_CORES)])
    out = full.transpose(0, 1, 4, 3, 2).reshape(16, 512, 512, 4)
    return np.ascontiguousarray(out).astype(np.float32)
